# revision 7
# baseline (speedup 1.0000x reference)
"""Trainium2 Bass kernel for nn_CrossAttention (B=4, C=256, H=W=64).

Per (batch, branch) the computation is an independent cross-attention:
    f = Wf @ other + bf          [32, 4096]
    g = Wg @ own   + bg          [32, 4096]
    h = Wh @ own   + bh          [256, 4096]
    S = f^T @ g                  [4096, 4096]
    att = softmax(S, axis=-1)
    sa[c, m] = sum_n h[c, n] * att[n, m]
    out = gamma * sa + own

B*2 = 8 independent problems -> one per NeuronCore (pure SPMD).

Factorization: att[n,m] = E[n,m]/Z[n] with E = exp(S - K0), Z = rowsum(E)
(accum_out of the exp activation), so sa = (h/Z)^T @ E with E computed once
in bf16.  The fixed K0 cancels in E/Z and guards fp32 overflow.

Schedule: 64 slots, one exp chunk [128n x 2048m] per slot (8 n-tile groups
x 8 chunks).  ACT streams exps back-to-back (the ~144us critical path);
the PE's trailing work each slot is sa accumulation for completed groups.
sa accumulates in PSUM across multi-group windows ({0,1},{2,3},{4}..{7})
before a single DVE eviction per (window, m-block, half) that also folds
gamma (STT: sa_sb += gamma*psum).  The residual add uses the fp16 input
(no fp32 copy of x is ever loaded).  E tiles and the oth input share one
rotating SBUF pool sized so exp never stalls on buffer reuse.
"""

import os
import sys

for _p in ("/opt/trn_rl_repo", "/opt/pypackages"):
    if _p not in sys.path:
        sys.path.insert(0, _p)

os.environ.setdefault("JAX_PLATFORMS", "")

import numpy as np

import concourse.bacc as bacc
import concourse.tile as tile
from concourse import mybir

F32 = mybir.dt.float32
F16 = mybir.dt.float16
BF16 = mybir.dt.bfloat16
AF = mybir.ActivationFunctionType
ALU = mybir.AluOpType

B, C, H, W = 4, 256, 64, 64
N = H * W            # 4096 pixels
C8 = C // 8          # 32
NT = N // 128        # 32 n-tiles
NGROUP = 4           # n-tiles per group (Z granularity)
NG = NT // NGROUP    # 8 groups
MB = 512             # m-block (one PSUM bank of fp32)
NMB = N // MB        # 8 m-blocks
HALF = 2048          # exp chunk columns (4 PSUM banks)
K0 = 40.0            # constant subtracted inside exp (cancels in softmax)
IN_T = 2048          # input tile columns
E_BUFS = 34          # rotating [128, 2048] pool: 4 oth tiles + 30 E tiles
# sa accumulation windows (groups whose contribution sums in PSUM before
# one eviction); later windows are single groups so their sa work lands
# inside the exp stream instead of after it.
WINDOWS = [[0, 1], [2, 3], [4], [5], [6], [7]]


def build_bass():
    nc = bacc.Bacc()

    own_d = nc.dram_tensor("own16", [C, N], F16, kind="ExternalInput")
    oth_d = nc.dram_tensor("oth16", [C, N], F16, kind="ExternalInput")
    wf_d = nc.dram_tensor("wf_t", [C, C8], F16, kind="ExternalInput")
    wg_d = nc.dram_tensor("wg_t", [C, C8], F16, kind="ExternalInput")
    wh_d = nc.dram_tensor("wh_t", [C, C], F16, kind="ExternalInput")
    bf_d = nc.dram_tensor("bf_rep", [128, 1], F32, kind="ExternalInput")
    bg_d = nc.dram_tensor("bg_rep", [128, 1], F32, kind="ExternalInput")
    bh_d = nc.dram_tensor("bh_row", [1, C], F16, kind="ExternalInput")
    gm_d = nc.dram_tensor("gamma_rep", [128, 1], F32, kind="ExternalInput")
    on_d = nc.dram_tensor("ones_row", [1, 128], F16, kind="ExternalInput")
    k0_d = nc.dram_tensor("k0_col", [128, 1], F32, kind="ExternalInput")
    out_d = nc.dram_tensor("out", [C, N], F32, kind="ExternalOutput")

    with tile.TileContext(nc) as tc:
        with (
            tc.tile_pool(name="singles", bufs=1) as singles,
            tc.tile_pool(name="own", bufs=1) as ownp,
            tc.tile_pool(name="big", bufs=E_BUFS) as big,
            tc.tile_pool(name="fp", bufs=8) as fpool,
            tc.tile_pool(name="zpool", bufs=4) as zpool,
            tc.tile_pool(name="outp", bufs=2) as outp,
            tc.tile_pool(name="ps_c", bufs=2, space="PSUM") as ps_c,
            tc.tile_pool(name="ps_s", bufs=1, space="PSUM") as ps_s,
            tc.tile_pool(name="ps_sa", bufs=2, space="PSUM") as ps_sa,
        ):
            # ---- small constants ----
            wf_sb = [singles.tile([128, C8], F16, name=f"wf{k}") for k in range(2)]
            wg_sb = [singles.tile([128, C8], F16, name=f"wg{k}") for k in range(2)]
            wh_sb = [singles.tile([128, C], F16, name=f"wh{k}") for k in range(2)]
            for k in range(2):
                nc.sync.dma_start(out=wf_sb[k], in_=wf_d[128 * k:128 * (k + 1), :])
                nc.sync.dma_start(out=wg_sb[k], in_=wg_d[128 * k:128 * (k + 1), :])
                nc.sync.dma_start(out=wh_sb[k], in_=wh_d[128 * k:128 * (k + 1), :])
            bf_sb = singles.tile([128, 1], F32)
            bg_sb = singles.tile([128, 1], F32)
            bh_sb = singles.tile([1, C], F16)
            gm_sb = singles.tile([128, 1], F32)
            ones_sb = singles.tile([1, 128], F16)
            k0_sb = singles.tile([128, 1], F32)
            nc.sync.dma_start(out=bf_sb, in_=bf_d[:, :])
            nc.sync.dma_start(out=bg_sb, in_=bg_d[:, :])
            nc.sync.dma_start(out=bh_sb, in_=bh_d[:, :])
            nc.sync.dma_start(out=gm_sb, in_=gm_d[:, :])
            nc.sync.dma_start(out=ones_sb, in_=on_d[:, :])
            nc.sync.dma_start(out=k0_sb, in_=k0_d[:, :])

            # inputs as [128, 2048] fp16 tiles: own (static), oth (rotating
            # through the big pool; recycled as E tiles once f convs finish).
            # DMA priority: everything the first stats chunk needs first.
            own_sb = [[ownp.tile([128, IN_T], F16, name=f"own{k}_{t}")
                       for t in range(2)] for k in range(2)]
            oth_sb = [[big.tile([128, IN_T], F16, name=f"oth{k}_{t}", tag="e")
                       for t in range(2)] for k in range(2)]
            for k in range(2):
                nc.sync.dma_start(out=own_sb[k][0], in_=own_d[128 * k:128 * (k + 1), 0:IN_T])
            for k in range(2):
                nc.sync.dma_start(out=oth_sb[k][0], in_=oth_d[128 * k:128 * (k + 1), 0:IN_T])
            for k in range(2):
                nc.sync.dma_start(out=own_sb[k][1], in_=own_d[128 * k:128 * (k + 1), IN_T:N])
            for k in range(2):
                nc.sync.dma_start(out=oth_sb[k][1], in_=oth_d[128 * k:128 * (k + 1), IN_T:N])

            # g blocks static (live for the whole kernel); f blocks rotate
            # (f_q[g] is only read during group g's stats chunks).
            g_q = [singles.tile([128, MB], F16, name=f"g{nb}") for nb in range(NMB)]
            f_q = {}
            sa_sb = [singles.tile([128, N], F16, name=f"sa{k}") for k in range(2)]
            hxz = [singles.tile([128, C], BF16, name=f"hxz{i}") for i in range(NT)]
            e_t = {}   # (g, a, h) -> E tile [128, HALF] bf16

            def conv_g(nb):
                ps = ps_c.tile([128, MB], F32, tag="c")
                for k in range(2):
                    nc.tensor.matmul(
                        out=ps[0:C8, :],
                        lhsT=wg_sb[k],
                        rhs=own_sb[k][nb // 4][:, MB * (nb % 4):MB * (nb % 4 + 1)],
                        start=(k == 0),
                        stop=(k == 1),
                    )
                nc.vector.tensor_scalar(
                    out=g_q[nb][0:C8, :], in0=ps[0:C8, :],
                    scalar1=bg_sb[0:C8, 0:1], scalar2=None, op0=ALU.add)
                for j in range(1, 4):
                    nc.sync.dma_start(out=g_q[nb][32 * j:32 * (j + 1), :],
                                      in_=g_q[nb][0:C8, :])

            def conv_f(nb):
                dst = fpool.tile([128, MB], F16, name=f"f{nb}", tag="f")
                f_q[nb] = dst
                ps = ps_c.tile([128, MB], F32, tag="c")
                for k in range(2):
                    nc.tensor.matmul(
                        out=ps[0:C8, :],
                        lhsT=wf_sb[k],
                        rhs=oth_sb[k][nb // 4][:, MB * (nb % 4):MB * (nb % 4 + 1)],
                        start=(k == 0),
                        stop=(k == 1),
                    )
                nc.vector.tensor_scalar(
                    out=dst[0:C8, :], in0=ps[0:C8, :],
                    scalar1=bf_sb[0:C8, 0:1], scalar2=None, op0=ALU.add)
                for j in range(1, 4):
                    nc.sync.dma_start(out=dst[32 * j:32 * (j + 1), :],
                                      in_=dst[0:C8, :])

            def conv_h(i):
                t, o = (128 * i) // IN_T, (128 * i) % IN_T
                ph = ps_c.tile([128, C], F32, tag="c")
                nc.tensor.matmul(out=ph, lhsT=ones_sb, rhs=bh_sb,
                                 start=True, stop=False)
                for k in range(2):
                    nc.tensor.matmul(
                        out=ph,
                        lhsT=own_sb[k][t][:, o:o + 128],
                        rhs=wh_sb[k],
                        start=False,
                        stop=(k == 1),
                    )
                nc.vector.tensor_copy(out=hxz[i], in_=ph)

            def stats_chunk(g, a, h, zp):
                """S chunk (n-tile 4g+a, m half h) -> exp -> E + Z part."""
                i = NGROUP * g + a
                o = 128 * (i % NGROUP)
                sp = ps_s.tile([128, HALF], F32, tag="s")
                for j in range(4):
                    nc.tensor.matmul(
                        out=sp[:, MB * j:MB * (j + 1)],
                        lhsT=f_q[g][32 * j:32 * (j + 1), o:o + 128],
                        rhs=g_q[4 * h + j][32 * j:32 * (j + 1), :],
                        start=True,
                        stop=True,
                        tile_position=(32 * j, 0),
                    )
                et = big.tile([128, HALF], BF16, name=f"e{g}_{a}_{h}", tag="e")
                e_t[(g, a, h)] = et
                nc.scalar.activation(
                    out=et,
                    in_=sp,
                    func=AF.Exp,
                    bias=k0_sb[:, 0:1],
                    accum_out=zp[:, 2 * a + h:2 * a + h + 1],
                )

            def zprep(g, zp):
                """Z = sum of the two half-sums; hxz *= 1/Z (in place)."""
                zt = zpool.tile([128, NGROUP], F32, tag="zt")
                rz = zpool.tile([128, NGROUP], F32, tag="rz")
                nc.vector.tensor_add(out=zt, in0=zp[:, 0:8:2], in1=zp[:, 1:8:2])
                nc.vector.reciprocal(out=rz, in_=zt)
                for a in range(NGROUP):
                    nc.vector.tensor_scalar(
                        out=hxz[NGROUP * g + a],
                        in0=hxz[NGROUP * g + a],
                        scalar1=rz[:, a:a + 1],
                        scalar2=None,
                        op0=ALU.mult,
                    )

            def sa_unit(w, mb, ch):
                """One window's contribution to sa[:, mb block], half ch."""
                h, m0 = mb // 4, MB * (mb % 4)
                groups = WINDOWS[w]
                pa = ps_sa.tile([128, MB], F32, tag="sa")
                nmm = 4 * len(groups)
                k = 0
                for g in groups:
                    for a in range(NGROUP):
                        nc.tensor.matmul(
                            out=pa,
                            lhsT=hxz[NGROUP * g + a][:, 128 * ch:128 * (ch + 1)],
                            rhs=e_t[(g, a, h)][:, m0:m0 + MB],
                            start=(k == 0),
                            stop=(k == nmm - 1),
                        )
                        k += 1
                dst = sa_sb[ch][:, MB * mb:MB * (mb + 1)]
                if w == 0:
                    nc.vector.tensor_scalar(
                        out=dst, in0=pa,
                        scalar1=gm_sb[:, 0:1], scalar2=None, op0=ALU.mult)
                else:
                    nc.vector.scalar_tensor_tensor(
                        out=dst, in0=pa, scalar=gm_sb[:, 0:1], in1=dst,
                        op0=ALU.mult, op1=ALU.add)
                if w == len(WINDOWS) - 1:
                    ot = outp.tile([128, MB], F32, tag="ot")
                    nc.vector.tensor_add(
                        out=ot, in0=dst,
                        in1=own_sb[ch][h][:, m0:m0 + MB])
                    nc.sync.dma_start(
                        out=out_d[128 * ch:128 * (ch + 1), MB * mb:MB * (mb + 1)],
                        in_=ot)

            # ---- slot schedule ----
            # upfront: what stats chunk (0,0,0) needs.
            conv_f(0)
            for nb in range(4):
                conv_g(nb)

            # early filler: remaining convs, ~3 per slot over the first 15
            # slots.  h convs for group g's n-tiles must land before
            # zprep(g) (slot 8g+8); all f convs must finish before E-tile
            # recycling reaches the oth slots of the big pool (slot ~30).
            filler = [("h", i) for i in range(4)] \
                   + [("g", nb) for nb in range(4, NMB)] \
                   + [("f", nb) for nb in range(1, 4)] \
                   + [("h", i) for i in range(4, 16)] \
                   + [("f", 4), ("f", 5)] \
                   + [("h", i) for i in range(16, 24)] \
                   + [("f", 6), ("f", 7)] \
                   + [("h", i) for i in range(24, NT)]
            FILL_PER_SLOT = 3

            # sa units become available per window after its last zprep.
            ready_slot = {w: 8 * (max(gs) + 1) for w, gs in enumerate(WINDOWS)}
            unit_queue = []
            for w in range(len(WINDOWS)):
                for mb in range(NMB):
                    for ch in range(2):
                        unit_queue.append((ready_slot[w], w, mb, ch))
            unit_queue.sort(key=lambda u: u[0])
            uq_pos = 0

            def emit_trailing(s, budget):
                """Emit trailing PE work for slot s."""
                nonlocal uq_pos
                for _ in range(FILL_PER_SLOT):
                    if filler:
                        kind, arg = filler.pop(0)
                        if kind == "g":
                            conv_g(arg)
                        elif kind == "f":
                            conv_f(arg)
                        else:
                            conv_h(arg)
                done = 0
                while done < budget and uq_pos < len(unit_queue):
                    rs, w, mb, ch = unit_queue[uq_pos]
                    if rs > s:
                        break
                    sa_unit(w, mb, ch)
                    uq_pos += 1
                    done += 1

            chunks = [(a, h) for h in range(2) for a in range(NGROUP)]
            zps = {}
            for g in range(NG):
                zps[g] = zpool.tile([128, 2 * NGROUP], F32, tag="zp", name=f"zp{g}")
                for k, (a, h) in enumerate(chunks):
                    s = 8 * g + k
                    if k == 0 and g > 0:
                        zprep(g - 1, zps.pop(g - 1))
                    emit_trailing(s, 1 if s < 32 else (2 if s < 48 else 3))
                    stats_chunk(g, a, h, zps[g])
            zprep(NG - 1, zps.pop(NG - 1))
            # tail: remaining units (last window)
            while uq_pos < len(unit_queue):
                _, w, mb, ch = unit_queue[uq_pos]
                sa_unit(w, mb, ch)
                uq_pos += 1

    if not nc.is_finalized():
        nc.finalize()
    return nc


_NC_CACHE = None


def _get_nc():
    global _NC_CACHE
    if _NC_CACHE is None:
        _NC_CACHE = build_bass()
    return _NC_CACHE


def make_in_maps(**inputs):
    """Build the 8 per-core input maps (core 2b = x-branch, 2b+1 = y-branch)."""
    f = lambda a: np.ascontiguousarray(np.asarray(a), dtype=np.float32)
    h16 = lambda a: np.ascontiguousarray(np.asarray(a), dtype=np.float16)
    x16 = h16(inputs["x"]).reshape(B, C, N)
    y16 = h16(inputs["y"]).reshape(B, C, N)
    Wfx, bfx = h16(inputs["Wfx"]), f(inputs["bfx"])
    Wgx, bgx = h16(inputs["Wgx"]), f(inputs["bgx"])
    Whx, bhx = h16(inputs["Whx"]), h16(inputs["bhx"])
    Wfy, bfy = h16(inputs["Wfy"]), f(inputs["bfy"])
    Wgy, bgy = h16(inputs["Wgy"]), f(inputs["bgy"])
    Why, bhy = h16(inputs["Why"]), h16(inputs["bhy"])
    gamma = f(inputs["gamma"])

    rep4 = lambda b: np.ascontiguousarray(np.tile(b, 4).reshape(128, 1))
    gam = np.ascontiguousarray(np.broadcast_to(gamma.reshape(1, 1), (128, 1)))

    c16 = lambda a: np.ascontiguousarray(a, dtype=np.float16)
    branch = {
        "x": dict(
            wf_t=c16(Wfy.T), wg_t=c16(Wgx.T), wh_t=c16(Whx.T),
            bf_rep=rep4(bfy), bg_rep=rep4(bgx), bh_row=c16(bhx.reshape(1, C)),
        ),
        "y": dict(
            wf_t=c16(Wfx.T), wg_t=c16(Wgy.T), wh_t=c16(Why.T),
            bf_rep=rep4(bfx), bg_rep=rep4(bgy), bh_row=c16(bhy.reshape(1, C)),
        ),
    }

    ones_row = np.ones((1, 128), np.float16)
    k0_col = np.full((128, 1), -K0, np.float32)
    in_maps = []
    for b in range(B):
        in_maps.append(dict(own16=x16[b], oth16=y16[b],
                            gamma_rep=gam, ones_row=ones_row, k0_col=k0_col,
                            **branch["x"]))
        in_maps.append(dict(own16=y16[b], oth16=x16[b],
                            gamma_rep=gam, ones_row=ones_row, k0_col=k0_col,
                            **branch["y"]))
    return in_maps


def kernel(**inputs):
    from concourse.bass_utils import run_bass_kernel_spmd

    nc = _get_nc()
    in_maps = make_in_maps(**inputs)
    res = run_bass_kernel_spmd(nc, in_maps, list(range(8))).results
    out_x = np.stack([res[2 * b]["out"] for b in range(B)]).reshape(B, C, H, W)
    out_y = np.stack([res[2 * b + 1]["out"] for b in range(B)]).reshape(B, C, H, W)
    return (out_x, out_y)


# revision 14
# speedup vs baseline: 1.0403x; 1.0403x over previous
"""Trainium2 Bass kernel for nn_CrossAttention (B=4, C=256, H=W=64).

Per (batch, branch) the computation is an independent cross-attention:
    f = Wf @ other + bf          [32, 4096]
    g = Wg @ own   + bg          [32, 4096]
    h = Wh @ own   + bh          [256, 4096]
    S = f^T @ g                  [4096, 4096]
    att = softmax(S, axis=-1)
    sa[c, m] = sum_n h[c, n] * att[n, m]
    out = gamma * sa + own

B*2 = 8 independent problems -> one per NeuronCore (pure SPMD).

Factorization: att[n,m] = E[n,m]/Z[n] with E = exp(S - K0), Z = rowsum(E)
(accum_out of the exp activation), so sa = (h/Z)^T @ E with E computed once
in bf16.  The fixed K0 cancels in E/Z and guards fp32 overflow.

Schedule: 64 slots, one exp chunk [128n x 2048m] per slot (8 n-tile groups
x 8 chunks).  ACT streams exps back-to-back (the ~144us critical path);
the PE's trailing work each slot is sa accumulation for completed groups.
sa accumulates in PSUM across multi-group windows ({0,1},{2,3},{4}..{7})
before a single DVE eviction per (window, m-block, half) that also folds
gamma (STT: sa_sb += gamma*psum).  The residual add uses the fp16 input
(no fp32 copy of x is ever loaded).  E tiles and the oth input share one
rotating SBUF pool sized so exp never stalls on buffer reuse.
"""

import os
import sys

for _p in ("/opt/trn_rl_repo", "/opt/pypackages"):
    if _p not in sys.path:
        sys.path.insert(0, _p)

os.environ.setdefault("JAX_PLATFORMS", "")

import numpy as np

import concourse.bacc as bacc
import concourse.tile as tile
from concourse import mybir

F32 = mybir.dt.float32
F16 = mybir.dt.float16
BF16 = mybir.dt.bfloat16
AF = mybir.ActivationFunctionType
ALU = mybir.AluOpType

B, C, H, W = 4, 256, 64, 64
N = H * W            # 4096 pixels
C8 = C // 8          # 32
NT = N // 128        # 32 n-tiles
NGROUP = 4           # n-tiles per group (Z granularity)
NG = NT // NGROUP    # 8 groups
MB = 512             # m-block (one PSUM bank of fp32)
NMB = N // MB        # 8 m-blocks
HALF = 2048          # exp chunk columns (4 PSUM banks)
K0 = 40.0            # constant subtracted inside exp (cancels in softmax)
IN_T = 2048          # input tile columns
E_BUFS = 30          # rotating [128, 2048] bf16 E tiles (3.75 groups live)
# sa accumulation windows (groups whose contribution sums in PSUM before
# one eviction); later windows are single groups so their sa work lands
# inside the exp stream instead of after it.
WINDOWS = [[0, 1], [2, 3], [4], [5], [6], [7]]


def build_bass():
    nc = bacc.Bacc()

    own_d = nc.dram_tensor("own16", [C, N], F16, kind="ExternalInput")
    oth_d = nc.dram_tensor("oth16", [C, N], F16, kind="ExternalInput")
    # wf/wg are pre-tiled 4x along their free dim so the conv matmul writes
    # all four partition-quad replicas directly (no SBUF-SBUF copy DMAs).
    wf_d = nc.dram_tensor("wf_t", [C, 128], F16, kind="ExternalInput")
    wg_d = nc.dram_tensor("wg_t", [C, 128], F16, kind="ExternalInput")
    wh_d = nc.dram_tensor("wh_t", [C, C], F16, kind="ExternalInput")
    bf_d = nc.dram_tensor("bf_rep", [128, 1], F32, kind="ExternalInput")
    bg_d = nc.dram_tensor("bg_rep", [128, 1], F32, kind="ExternalInput")
    bh_d = nc.dram_tensor("bh_row", [1, C], F16, kind="ExternalInput")
    gm_d = nc.dram_tensor("gamma_rep", [128, 1], F32, kind="ExternalInput")
    on_d = nc.dram_tensor("ones_row", [1, 128], F16, kind="ExternalInput")
    k0_d = nc.dram_tensor("k0_col", [128, 1], F32, kind="ExternalInput")
    out_d = nc.dram_tensor("out", [C, N], F32, kind="ExternalOutput")

    with tile.TileContext(nc) as tc:
        with (
            tc.tile_pool(name="singles", bufs=1) as singles,
            tc.tile_pool(name="own", bufs=1) as ownp,
            tc.tile_pool(name="othp", bufs=1) as othp,
            tc.tile_pool(name="big", bufs=E_BUFS) as big,
            tc.tile_pool(name="fp", bufs=8) as fpool,
            tc.tile_pool(name="zpool", bufs=4) as zpool,
            tc.tile_pool(name="outp", bufs=2) as outp,
            tc.tile_pool(name="ps_c", bufs=2, space="PSUM") as ps_c,
            tc.tile_pool(name="ps_s", bufs=1, space="PSUM") as ps_s,
            tc.tile_pool(name="ps_sa", bufs=2, space="PSUM") as ps_sa,
        ):
            # ---- DMA priority: what the first convs need comes first ----
            wf_sb = [singles.tile([128, 128], F16, name=f"wf{k}") for k in range(2)]
            wg_sb = [singles.tile([128, 128], F16, name=f"wg{k}") for k in range(2)]
            wh_sb = [singles.tile([128, C], F16, name=f"wh{k}") for k in range(2)]
            bf_sb = singles.tile([128, 1], F32)
            bg_sb = singles.tile([128, 1], F32)
            bh_sb = singles.tile([1, C], F16)
            gm_sb = singles.tile([128, 1], F32)
            ones_sb = singles.tile([1, 128], F16)
            k0_sb = singles.tile([128, 1], F32)
            own_sb = [[ownp.tile([128, IN_T], F16, name=f"own{k}_{t}")
                       for t in range(2)] for k in range(2)]
            oth_sb = [[othp.tile([128, IN_T], F16, name=f"oth{k}_{t}")
                       for t in range(2)] for k in range(2)]

            for k in range(2):
                nc.sync.dma_start(out=wg_sb[k], in_=wg_d[128 * k:128 * (k + 1), :])
                nc.sync.dma_start(out=wf_sb[k], in_=wf_d[128 * k:128 * (k + 1), :])
            nc.sync.dma_start(out=bg_sb, in_=bg_d[:, :])
            nc.sync.dma_start(out=bf_sb, in_=bf_d[:, :])
            for k in range(2):
                nc.sync.dma_start(out=own_sb[k][0], in_=own_d[128 * k:128 * (k + 1), 0:IN_T])
            for k in range(2):
                nc.sync.dma_start(out=oth_sb[k][0], in_=oth_d[128 * k:128 * (k + 1), 0:IN_T])
            nc.sync.dma_start(out=k0_sb, in_=k0_d[:, :])
            for k in range(2):
                nc.sync.dma_start(out=wh_sb[k], in_=wh_d[128 * k:128 * (k + 1), :])
            nc.sync.dma_start(out=bh_sb, in_=bh_d[:, :])
            nc.sync.dma_start(out=ones_sb, in_=on_d[:, :])
            nc.sync.dma_start(out=gm_sb, in_=gm_d[:, :])
            for k in range(2):
                nc.sync.dma_start(out=own_sb[k][1], in_=own_d[128 * k:128 * (k + 1), IN_T:N])
            for k in range(2):
                nc.sync.dma_start(out=oth_sb[k][1], in_=oth_d[128 * k:128 * (k + 1), IN_T:N])

            # g blocks static (live for the whole kernel); f blocks rotate
            # (f_q[g] is only read during group g's stats chunks).
            g_q = [singles.tile([128, MB], F16, name=f"g{nb}") for nb in range(NMB)]
            f_q = {}
            sa_sb = [singles.tile([128, N], F16, name=f"sa{k}") for k in range(2)]
            hxz = [singles.tile([128, C], BF16, name=f"hxz{i}") for i in range(NT)]
            e_t = {}   # (g, a, h) -> E tile [128, HALF] bf16

            def conv_g(nb):
                ps = ps_c.tile([128, MB], F32, tag="c")
                for k in range(2):
                    nc.tensor.matmul(
                        out=ps,
                        lhsT=wg_sb[k],
                        rhs=own_sb[k][nb // 4][:, MB * (nb % 4):MB * (nb % 4 + 1)],
                        start=(k == 0),
                        stop=(k == 1),
                    )
                nc.vector.tensor_scalar(
                    out=g_q[nb], in0=ps,
                    scalar1=bg_sb[:, 0:1], scalar2=None, op0=ALU.add)

            def conv_f(nb):
                dst = fpool.tile([128, MB], F16, name=f"f{nb}", tag="f")
                f_q[nb] = dst
                ps = ps_c.tile([128, MB], F32, tag="c")
                for k in range(2):
                    nc.tensor.matmul(
                        out=ps,
                        lhsT=wf_sb[k],
                        rhs=oth_sb[k][nb // 4][:, MB * (nb % 4):MB * (nb % 4 + 1)],
                        start=(k == 0),
                        stop=(k == 1),
                    )
                nc.vector.tensor_scalar(
                    out=dst, in0=ps,
                    scalar1=bf_sb[:, 0:1], scalar2=None, op0=ALU.add)

            def conv_h(i):
                t, o = (128 * i) // IN_T, (128 * i) % IN_T
                ph = ps_c.tile([128, C], F32, tag="c")
                nc.tensor.matmul(out=ph, lhsT=ones_sb, rhs=bh_sb,
                                 start=True, stop=False)
                for k in range(2):
                    nc.tensor.matmul(
                        out=ph,
                        lhsT=own_sb[k][t][:, o:o + 128],
                        rhs=wh_sb[k],
                        start=False,
                        stop=(k == 1),
                    )
                nc.vector.tensor_copy(out=hxz[i], in_=ph)

            def stats_chunk(g, a, h, zp):
                """S chunk (n-tile 4g+a, m half h) -> exp -> E + Z part."""
                i = NGROUP * g + a
                o = 128 * (i % NGROUP)
                sp = ps_s.tile([128, HALF], F32, tag="s")
                for j in range(4):
                    nc.tensor.matmul(
                        out=sp[:, MB * j:MB * (j + 1)],
                        lhsT=f_q[g][32 * j:32 * (j + 1), o:o + 128],
                        rhs=g_q[4 * h + j][32 * j:32 * (j + 1), :],
                        start=True,
                        stop=True,
                        tile_position=(32 * j, 0),
                    )
                et = big.tile([128, HALF], BF16, name=f"e{g}_{a}_{h}", tag="e")
                e_t[(g, a, h)] = et
                nc.scalar.activation(
                    out=et,
                    in_=sp,
                    func=AF.Exp,
                    bias=k0_sb[:, 0:1],
                    accum_out=zp[:, 2 * a + h:2 * a + h + 1],
                )

            def zprep(g, zp):
                """Z = sum of the two half-sums; hxz *= 1/Z (in place)."""
                zt = zpool.tile([128, NGROUP], F32, tag="zt")
                rz = zpool.tile([128, NGROUP], F32, tag="rz")
                nc.vector.tensor_add(out=zt, in0=zp[:, 0:8:2], in1=zp[:, 1:8:2])
                nc.vector.reciprocal(out=rz, in_=zt)
                for a in range(NGROUP):
                    nc.vector.tensor_scalar(
                        out=hxz[NGROUP * g + a],
                        in0=hxz[NGROUP * g + a],
                        scalar1=rz[:, a:a + 1],
                        scalar2=None,
                        op0=ALU.mult,
                    )

            def sa_unit(w, mb, ch):
                """One window's contribution to sa[:, mb block], half ch."""
                h, m0 = mb // 4, MB * (mb % 4)
                groups = WINDOWS[w]
                pa = ps_sa.tile([128, MB], F32, tag="sa")
                nmm = 4 * len(groups)
                k = 0
                for g in groups:
                    for a in range(NGROUP):
                        nc.tensor.matmul(
                            out=pa,
                            lhsT=hxz[NGROUP * g + a][:, 128 * ch:128 * (ch + 1)],
                            rhs=e_t[(g, a, h)][:, m0:m0 + MB],
                            start=(k == 0),
                            stop=(k == nmm - 1),
                        )
                        k += 1
                dst = sa_sb[ch][:, MB * mb:MB * (mb + 1)]
                if w == 0:
                    nc.vector.tensor_scalar(
                        out=dst, in0=pa,
                        scalar1=gm_sb[:, 0:1], scalar2=None, op0=ALU.mult)
                else:
                    nc.vector.scalar_tensor_tensor(
                        out=dst, in0=pa, scalar=gm_sb[:, 0:1], in1=dst,
                        op0=ALU.mult, op1=ALU.add)
                if w == len(WINDOWS) - 1:
                    ot = outp.tile([128, MB], F32, tag="ot")
                    nc.vector.tensor_add(
                        out=ot, in0=dst,
                        in1=own_sb[ch][h][:, m0:m0 + MB])
                    nc.sync.dma_start(
                        out=out_d[128 * ch:128 * (ch + 1), MB * mb:MB * (mb + 1)],
                        in_=ot)

            # ---- slot schedule ----
            # upfront: what stats chunk (0,0,0) needs.
            conv_f(0)
            for nb in range(4):
                conv_g(nb)

            # early filler: remaining convs, ~3 per slot over the first 15
            # slots.  h convs for group g's n-tiles must land before
            # zprep(g) (slot 8g+8); all f convs must finish before E-tile
            # recycling reaches the oth slots of the big pool (slot ~30).
            filler = [("h", i) for i in range(4)] \
                   + [("g", nb) for nb in range(4, NMB)] \
                   + [("f", nb) for nb in range(1, 4)] \
                   + [("h", i) for i in range(4, 16)] \
                   + [("f", 4), ("f", 5)] \
                   + [("h", i) for i in range(16, 24)] \
                   + [("f", 6), ("f", 7)] \
                   + [("h", i) for i in range(24, NT)]
            FILL_PER_SLOT = 4

            # sa units become available per window after its last zprep.
            ready_slot = {w: 8 * (max(gs) + 1) for w, gs in enumerate(WINDOWS)}
            unit_queue = []
            for w in range(len(WINDOWS)):
                for mb in range(NMB):
                    for ch in range(2):
                        unit_queue.append((ready_slot[w], w, mb, ch))
            unit_queue.sort(key=lambda u: u[0])
            uq_pos = 0

            def emit_trailing(s, budget):
                """Emit trailing PE work for slot s."""
                nonlocal uq_pos
                for _ in range(FILL_PER_SLOT):
                    if filler:
                        kind, arg = filler.pop(0)
                        if kind == "g":
                            conv_g(arg)
                        elif kind == "f":
                            conv_f(arg)
                        else:
                            conv_h(arg)
                done = 0
                while done < budget and uq_pos < len(unit_queue):
                    rs, w, mb, ch = unit_queue[uq_pos]
                    if rs > s:
                        break
                    sa_unit(w, mb, ch)
                    uq_pos += 1
                    done += 1

            chunks = [(a, h) for h in range(2) for a in range(NGROUP)]
            zps = {}
            for g in range(NG):
                zps[g] = zpool.tile([128, 2 * NGROUP], F32, tag="zp", name=f"zp{g}")
                for k, (a, h) in enumerate(chunks):
                    s = 8 * g + k
                    if k == 0 and g > 0:
                        zprep(g - 1, zps.pop(g - 1))
                    emit_trailing(s, 1 if s < 32 else (2 if s < 48 else 3))
                    stats_chunk(g, a, h, zps[g])
            zprep(NG - 1, zps.pop(NG - 1))
            # tail: remaining units (last window)
            while uq_pos < len(unit_queue):
                _, w, mb, ch = unit_queue[uq_pos]
                sa_unit(w, mb, ch)
                uq_pos += 1

    if not nc.is_finalized():
        nc.finalize()
    return nc


_NC_CACHE = None


def _get_nc():
    global _NC_CACHE
    if _NC_CACHE is None:
        _NC_CACHE = build_bass()
    return _NC_CACHE


def make_in_maps(**inputs):
    """Build the 8 per-core input maps (core 2b = x-branch, 2b+1 = y-branch)."""
    f = lambda a: np.ascontiguousarray(np.asarray(a), dtype=np.float32)
    h16 = lambda a: np.ascontiguousarray(np.asarray(a), dtype=np.float16)
    x16 = h16(inputs["x"]).reshape(B, C, N)
    y16 = h16(inputs["y"]).reshape(B, C, N)
    Wfx, bfx = h16(inputs["Wfx"]), f(inputs["bfx"])
    Wgx, bgx = h16(inputs["Wgx"]), f(inputs["bgx"])
    Whx, bhx = h16(inputs["Whx"]), h16(inputs["bhx"])
    Wfy, bfy = h16(inputs["Wfy"]), f(inputs["bfy"])
    Wgy, bgy = h16(inputs["Wgy"]), f(inputs["bgy"])
    Why, bhy = h16(inputs["Why"]), h16(inputs["bhy"])
    gamma = f(inputs["gamma"])

    rep4 = lambda b: np.ascontiguousarray(np.tile(b, 4).reshape(128, 1))
    gam = np.ascontiguousarray(np.broadcast_to(gamma.reshape(1, 1), (128, 1)))

    c16 = lambda a: np.ascontiguousarray(a, dtype=np.float16)
    rep4c = lambda w: c16(np.tile(w.T, (1, 4)))   # [C, C8] -> [C, 128]
    branch = {
        "x": dict(
            wf_t=rep4c(Wfy), wg_t=rep4c(Wgx), wh_t=c16(Whx.T),
            bf_rep=rep4(bfy), bg_rep=rep4(bgx), bh_row=c16(bhx.reshape(1, C)),
        ),
        "y": dict(
            wf_t=rep4c(Wfx), wg_t=rep4c(Wgy), wh_t=c16(Why.T),
            bf_rep=rep4(bfx), bg_rep=rep4(bgy), bh_row=c16(bhy.reshape(1, C)),
        ),
    }

    ones_row = np.ones((1, 128), np.float16)
    k0_col = np.full((128, 1), -K0, np.float32)
    in_maps = []
    for b in range(B):
        in_maps.append(dict(own16=x16[b], oth16=y16[b],
                            gamma_rep=gam, ones_row=ones_row, k0_col=k0_col,
                            **branch["x"]))
        in_maps.append(dict(own16=y16[b], oth16=x16[b],
                            gamma_rep=gam, ones_row=ones_row, k0_col=k0_col,
                            **branch["y"]))
    return in_maps


def kernel(**inputs):
    from concourse.bass_utils import run_bass_kernel_spmd

    nc = _get_nc()
    in_maps = make_in_maps(**inputs)
    res = run_bass_kernel_spmd(nc, in_maps, list(range(8))).results
    out_x = np.stack([res[2 * b]["out"] for b in range(B)]).reshape(B, C, H, W)
    out_y = np.stack([res[2 * b + 1]["out"] for b in range(B)]).reshape(B, C, H, W)
    return (out_x, out_y)
